# revision 1
# baseline (speedup 1.0000x reference)
"""Bahdanau-attention kernel for one TRN2 chip (8 NeuronCores, SPMD).

Math (per batch row b, sequence position s):
    att[b, s] = v . tanh(h_part[b] + enc[s, b, :] @ W_e)
    out[b, :] = softmax(att[b, :])        with h_part = hidden @ W_h + b_attn

Sharding: pure data-parallel over batch (B=32 -> 4 per core), no collectives.

Key design points:
- Host-side layout prep: the big matmul contracts over H, which must live on
  SBUF partitions, so encoder_outputs is pre-transposed to H-major on the host
  and every device DMA is one contiguous block.
- The energy matmul runs as fp8(e4m3) DoubleRow (2 weights/cell, effective
  K=256 per pass, half the matmul count of bf16).  W_e is pre-scaled by 64 on
  the host so its small values stay in fp8's normal range; the tanh activation
  rescales by 1/64 for free.  h_part / v-dot stay bf16; accumulation is fp32.
- tanh(h_part + e_part) runs on the scalar engine with the per-(q,b) bias
  folded in; [128,1024] tiles halve the per-op overhead.  Softmax skips the
  max-subtraction (|logit| <= ||v||_1 ~ 18, safe in fp32 exp).
- Software-pipelined emission: e-matmuls of block i+1 precede the
  tanh-dependent v-dot matmuls of block i-1 in the PE stream (2-block skew),
  exp is deferred one block so it never head-of-line-blocks tanh in the ACT
  FIFO, and dummy matmuls pre-warm the PE clock (HAM) during the first DMAs.
Measured: ~78 us on-chip (neuron-profile exec_time), rel err ~1.3e-2 vs the
fp32 reference (L2); max abs err ~6e-5 on a softmax output of scale ~0.1.
"""

import sys

sys.path.insert(0, "/opt/trn_rl_repo")

import numpy as np

from concourse import bacc, bass, mybir, tile
from concourse.bass_utils import run_bass_kernel_spmd

H = 512
DH = 4 * H            # 2048 (hidden feature dim)
B, S = 32, 2048
NCORES = 8
BC = B // NCORES      # 4 batch rows per core
KH = H // 128         # 4 contraction tiles over H
KD = DH // 128        # 16 contraction tiles over DH
NQ = H // 128         # 4 output quadrants of H
SBLK = 1024           # sequence positions per block
NBLK = S // SBLK      # 2 blocks per batch row
HB = 512              # half-block: psum-bank / matmul-N granularity
NCH = S // HB         # 4 per-row chunks for the softmax
F32 = mybir.dt.float32
F32R = mybir.dt.float32r
BF16 = mybir.dt.bfloat16
F8 = mybir.dt.float8e4
WE_SCALE = 64.0

_NC_CACHE = None


def _build():
    nc = bacc.Bacc(
        "TRN2", target_bir_lowering=False, debug=False, num_devices=NCORES
    )
    enc_d = nc.dram_tensor(
        "enc_t", [BC, NBLK, 128, KH, SBLK], F8, kind="ExternalInput"
    )
    hid_d = nc.dram_tensor("hid_t", [128, KD, BC], BF16, kind="ExternalInput")
    wh_d = nc.dram_tensor("w_h", [128, KD, H], BF16, kind="ExternalInput")
    we_d = nc.dram_tensor("w_e", [128, KH, H], F8, kind="ExternalInput")
    ba_d = nc.dram_tensor("b_attn", [128, NQ], F32, kind="ExternalInput")
    v_d = nc.dram_tensor("v", [128, NQ], BF16, kind="ExternalInput")
    id_d = nc.dram_tensor("ident", [BC, BC], F32, kind="ExternalInput")
    out_d = nc.dram_tensor("out", [BC, S], F32, kind="ExternalOutput")

    TANH = mybir.ActivationFunctionType.Tanh
    EXP = mybir.ActivationFunctionType.Exp
    COPY = mybir.ActivationFunctionType.Copy

    with tile.TileContext(nc) as tc:
        with (
            tc.tile_pool(name="const", bufs=1) as constp,
            tc.tile_pool(name="enc", bufs=6) as encp,
            tc.tile_pool(name="energy", bufs=8) as enp,
            tc.tile_pool(name="small", bufs=1) as smallp,
            tc.tile_pool(name="psum_e", bufs=3, space=bass.MemorySpace.PSUM) as pse,
            tc.tile_pool(name="psum_s", bufs=1, space=bass.MemorySpace.PSUM) as pss,
        ):
            wh_sb = constp.tile([128, KD, H], BF16)
            nc.scalar.dma_start(wh_sb[:, 0 : KD // 2, :], wh_d[:, 0 : KD // 2, :])
            we_sb = constp.tile([128, KH, H], F8)
            for k in range(KH):
                nc.scalar.dma_start(we_sb[:, k, :], we_d[:, k, :])
            ba_sb = constp.tile([128, NQ], F32)
            nc.scalar.dma_start(ba_sb[:], ba_d[:])
            v_sb = constp.tile([128, NQ], BF16)
            nc.scalar.dma_start(v_sb[:], v_d[:])
            id_sb = constp.tile([BC, BC], F32)
            nc.scalar.dma_start(id_sb[:], id_d[:])

            hptb = constp.tile([128, NQ, BC], F32)
            ex = smallp.tile([128, S], F32)
            out_sb = smallp.tile([128, S], F32)
            esum = smallp.tile([128, NCH], F32)
            ssum = smallp.tile([128, 1], F32)
            rs = smallp.tile([128, 1], F32)

            ps_small = pss.tile([128, HB], F32)

            # HAM pre-warm: ~3.5 us of dummy matmuls on zeroed scratch while
            # the first DMAs are still in flight, so real matmuls start at
            # full clock (K=8/8)
            warm = constp.tile([128, 512], BF16)
            nc.vector.memset(warm[:], 0.0)
            for _ in range(8):
                nc.tensor.matmul(
                    ps_small[:, :], warm[:, 0:128], warm[:], start=True, stop=True
                )

            blocks = [(b, s) for b in range(BC) for s in range(NBLK)]
            NBLOCKS = len(blocks)
            ets = {}
            epss = {}

            def load_block(i):
                b, sblk = blocks[i]
                et = encp.tile([128, KH, SBLK], F8)
                nc.sync.dma_start(et[:], enc_d[b, sblk])
                ets[i] = et

            def emit_emm(i, qs=None):
                b, sblk = blocks[i]
                if qs is None or qs[0] == 0:
                    epss[i] = []
                et = ets[i]
                eps4 = epss[i]
                qlist = list(qs) if qs is not None else list(range(NQ))
                tiles = {}
                for q in qlist:
                    tiles[q] = pse.tile([128, SBLK], F32, name="eps", tag="eps")
                for qpair in [qlist[i : i + 2] for i in range(0, len(qlist), 2)]:
                    for half in range(SBLK // HB):
                        hsl = slice(half * HB, (half + 1) * HB)
                        for j in range(KH // 2):
                            for q in qpair:
                                nc.tensor.matmul(
                                    tiles[q][:, hsl],
                                    we_sb[
                                        :, 2 * j : 2 * j + 2, q * 128 : (q + 1) * 128
                                    ],
                                    et[:, 2 * j : 2 * j + 2, hsl],
                                    start=(j == 0),
                                    stop=(j == KH // 2 - 1),
                                    perf_mode=mybir.MatmulPerfMode.DoubleRow,
                                )
                for q in qlist:
                    eps4.append(tiles[q])
                if qs is None or qs[-1] == NQ - 1:
                    ets.pop(i)

            ens = {}

            def emit_tanh(i):
                b, sblk = blocks[i]
                en4 = []
                for q in range(NQ):
                    eps = epss[i][q]
                    en = enp.tile([128, SBLK], BF16)
                    nc.scalar.activation(
                        en[:],
                        eps[:],
                        TANH,
                        bias=hptb[:, q, b : b + 1],
                        scale=1.0 / WE_SCALE,
                    )
                    en4.append(en)
                ens[i] = en4
                del epss[i]

            def emit_v(i):
                for half in range(SBLK // HB):
                    c = i * (SBLK // HB) + half
                    att_ps = ps_small[(c % 3) * 32 : (c % 3) * 32 + 1, 0:HB]
                    for q in range(NQ):
                        nc.tensor.matmul(
                            att_ps,
                            v_sb[:, q : q + 1],
                            ens[i][q][:, half * HB : (half + 1) * HB],
                            start=(q == 0),
                            stop=(q == NQ - 1),
                        )
                del ens[i]

            def emit_exp(i):
                # exp of block i's logits (no max-sub: |logit| <= ||v||_1 ~ 18).
                # Deferred so it never head-of-line-blocks tanh in the ACT FIFO.
                b, sblk = blocks[i]
                r0 = b * 32
                for half in range(SBLK // HB):
                    emit_exp_chunk(i, b, sblk * (SBLK // HB) + half,
                                   i * (SBLK // HB) + half)
                if sblk == NBLK - 1:
                    emit_norm(i, b, r0)

            def emit_exp_chunk(i, b, c, cg):
                r0 = b * 32
                att_ps = ps_small[(cg % 3) * 32 : (cg % 3) * 32 + 1, 0:HB]
                if i >= NBLOCKS - 2:
                    # tail-critical: fused accumulator (290 ns) beats a
                    # separate 680 ns single-partition DVE reduce
                    nc.scalar.activation(
                        ex[r0 : r0 + 1, c * HB : (c + 1) * HB],
                        att_ps,
                        EXP,
                        accum_out=esum[r0 : r0 + 1, c : c + 1],
                    )
                else:
                    nc.scalar.activation(
                        ex[r0 : r0 + 1, c * HB : (c + 1) * HB],
                        att_ps,
                        EXP,
                    )
                    nc.vector.reduce_sum(
                        esum[r0 : r0 + 1, c : c + 1],
                        ex[r0 : r0 + 1, c * HB : (c + 1) * HB],
                        axis=mybir.AxisListType.X,
                    )

            def emit_norm(i, b, r0):
                if True:
                    # normalize row b as soon as its blocks are done
                    nc.vector.reduce_sum(
                        ssum[r0 : r0 + 1, :],
                        esum[r0 : r0 + 1, :],
                        axis=mybir.AxisListType.X,
                    )
                    nc.vector.reciprocal(rs[r0 : r0 + 1, :], ssum[r0 : r0 + 1, :])
                    if i == NBLOCKS - 1:
                        # last row: split across engines so the exposed tail
                        # is half as long
                        hs = S // 2
                        nc.vector.tensor_scalar_mul(
                            out_sb[r0 : r0 + 1, 0:hs],
                            ex[r0 : r0 + 1, 0:hs],
                            rs[r0 : r0 + 1, :],
                        )
                        nc.scalar.activation(
                            out_sb[r0 : r0 + 1, hs:S],
                            ex[r0 : r0 + 1, hs:S],
                            COPY,
                            scale=rs[r0 : r0 + 1, :],
                        )
                        nc.sync.dma_start(
                            out_d[b : b + 1, 0:hs], out_sb[r0 : r0 + 1, 0:hs]
                        )
                        nc.scalar.dma_start(
                            out_d[b : b + 1, hs:S], out_sb[r0 : r0 + 1, hs:S]
                        )
                    else:
                        nc.vector.tensor_scalar_mul(
                            out_sb[r0 : r0 + 1, :],
                            ex[r0 : r0 + 1, :],
                            rs[r0 : r0 + 1, :],
                        )
                        nc.sync.dma_start(
                            out_d[b : b + 1, :], out_sb[r0 : r0 + 1, :]
                        )

            # prologue: sync queue carries only enc tiles (fp8, 256 KB each);
            # h_part matmuls interleave with block 0's e-matmuls so the tanh
            # bias is ready as early as possible
            load_block(0)
            hid_sb = constp.tile([128, KD, BC], BF16)
            nc.sync.dma_start(hid_sb[:], hid_d[:])
            nc.sync.dma_start(wh_sb[:, KD // 2 :, :], wh_d[:, KD // 2 :, :])
            load_block(1)
            hp_ps = ps_small[0:BC, 0:H]

            def emit_hp(ks):
                for k in ks:
                    nc.tensor.matmul(
                        hp_ps,
                        hid_sb[:, k, :],
                        wh_sb[:, k, :],
                        start=(k == 0),
                        stop=(k == KD - 1),
                    )

            emit_hp(range(KD))
            hp_sb = smallp.tile([BC, H], F32)
            nc.vector.tensor_copy(hp_sb[:], hp_ps)

            # transpose to [128, q, b] via PE, fold in b_attn -> tanh bias
            for q in range(NQ):
                hpt_ps = ps_small[:, q * BC : (q + 1) * BC]
                nc.tensor.transpose(
                    hpt_ps, hp_sb[:, q * 128 : (q + 1) * 128], id_sb[:]
                )
                nc.vector.tensor_scalar_add(
                    hptb[:, q, :], hpt_ps, ba_sb[:, q : q + 1]
                )
            emit_emm(0)

            # steady state, one-block skew: e-matmuls of block i+1 sit ahead of
            # block i's tanh-dependent v-dots in the PE stream
            for i in range(NBLOCKS):
                if i + 2 < NBLOCKS:
                    load_block(i + 2)
                if i + 1 < NBLOCKS:
                    emit_emm(i + 1)
                emit_tanh(i)
                if i >= 1:
                    emit_v(i - 1)
                    emit_exp(i - 1)
            emit_v(NBLOCKS - 1)
            emit_exp(NBLOCKS - 1)

    nc.compile()
    return nc


def _get_nc():
    global _NC_CACHE
    if _NC_CACHE is None:
        _NC_CACHE = _build()
    return _NC_CACHE


def _prep_inputs(hidden, encoder_outputs, W_attn, b_attn, v):
    f = np.float32
    W_h = np.asarray(W_attn[:DH], dtype=f)
    W_e = np.asarray(W_attn[DH:], dtype=f)
    import ml_dtypes
    bf = ml_dtypes.bfloat16
    f8 = ml_dtypes.float8_e4m3
    wh_prep = np.ascontiguousarray(W_h.reshape(KD, 128, H).transpose(1, 0, 2)).astype(bf)
    we_prep = np.clip(
        np.ascontiguousarray(W_e.reshape(KH, 128, H).transpose(1, 0, 2)) * 64.0,
        -240.0, 240.0,
    ).astype(f8)
    ba_prep = np.ascontiguousarray(np.asarray(b_attn, dtype=f).reshape(NQ, 128).T)
    v_prep = np.ascontiguousarray(np.asarray(v, dtype=f).reshape(NQ, 128).T).astype(bf)
    ident = np.eye(BC, dtype=f)
    hidden = np.asarray(hidden, dtype=f)
    encoder_outputs = np.asarray(encoder_outputs, dtype=f)

    in_maps = []
    for c in range(NCORES):
        b0 = c * BC
        hc = hidden[b0 : b0 + BC]                       # [BC, DH]
        hid_prep = np.ascontiguousarray(
            hc.T.reshape(KD, 128, BC).transpose(1, 0, 2)
        ).astype(bf)
        ec = encoder_outputs[:, b0 : b0 + BC, :]        # [S, BC, H]
        # enc_prep[b, sblk, p, k, si] = ec[sblk*SBLK+si, b, k*128+p]
        enc_prep = np.clip(
            np.ascontiguousarray(
                ec.transpose(1, 0, 2)
                .reshape(BC, NBLK, SBLK, KH, 128)
                .transpose(0, 1, 4, 3, 2)
            ),
            -240.0, 240.0,
        ).astype(ml_dtypes.float8_e4m3)
        in_maps.append(
            {
                "enc_t": enc_prep,
                "hid_t": hid_prep,
                "w_h": wh_prep,
                "w_e": we_prep,
                "b_attn": ba_prep,
                "v": v_prep,
                "ident": ident,
            }
        )
    return in_maps


def _run(inputs, trace=False, **kw):
    nc = _get_nc()
    in_maps = _prep_inputs(
        inputs["hidden"],
        inputs["encoder_outputs"],
        inputs["W_attn"],
        inputs["b_attn"],
        inputs["v"],
    )
    res = run_bass_kernel_spmd(
        nc, in_maps, core_ids=list(range(NCORES)), trace=trace, **kw
    )
    out = np.concatenate([r["out"] for r in res.results], axis=0).astype(np.float32)
    return out, res


def kernel(**inputs):
    out, _ = _run(inputs, trace=False)
    return out



# revision 9
# speedup vs baseline: 1.0216x; 1.0216x over previous
"""Bahdanau-attention kernel for one TRN2 chip (8 NeuronCores, SPMD).

Math (per batch row b, sequence position s):
    att[b, s] = v . tanh(h_part[b] + enc[s, b, :] @ W_e)
    out[b, :] = softmax(att[b, :])        with h_part = hidden @ W_h + b_attn

Sharding: pure data-parallel over batch (B=32 -> 4 per core), no collectives.

v2 design notes (from the v1 trace: 79.6us, ACT busy 63us, first ACT op at
t=24.5us, PE at half clock for the last 16us):
- Prologue cut from 24.5us to ~5us: weights arrive on parallel DMA queues
  (sync/vector/gpsimd), h_part runs as fp8 DoubleRow (8 matmuls) right after
  a short PE clock warm-up, and tanh of block 0 is gated only on its own
  e-matmul + the h_part transpose chain.
- s-major block order (all 4 batch rows at seq-block 0, then seq-block 1).
- The v-dot for (row b, 512-chunk c) uses a zero-padded stationary [128, 32]
  with v in column c, so its logit lands on psum partition 32b+c.  All 16
  (b, c) logit vectors accumulate into ONE [128, 512] psum bank; strips are
  per-b accumulation groups spanning all 4 chunks (zeros elsewhere add 0).
- Softmax then collapses to: ONE [128, 512] exp (+accum row sums), a [128,4]
  selection matmul for per-row sums, reciprocal, a broadcast-back matmul to
  per-partition scalars, one DVE tensor_scalar multiply, and ONE [128, 512]
  output DMA (2KB/partition instead of 8KB on a single partition).
- The ACT queue carries only 32 tanh + 1 exp; all DMAs are issued from
  sync/vector/gpsimd queues (v1 burned 6.5us of ACT time on DMA_DIRECT2D),
  and softmax reductions/normalization run on DVE/PE, not ACT.
Measured v1: 79.6us.  This file: see test output.
"""

import sys

sys.path.insert(0, "/opt/trn_rl_repo")

import numpy as np

from concourse import bacc, bass, mybir, tile
from concourse.bass_utils import run_bass_kernel_spmd

H = 512
DH = 4 * H            # 2048 (hidden feature dim)
B, S = 32, 2048
NCORES = 8
BC = B // NCORES      # 4 batch rows per core
KH = H // 128         # 4 contraction tiles over H
KD = DH // 128        # 16 contraction tiles over DH
NQ = H // 128         # 4 output quadrants of H
SBLK = 1024           # sequence positions per block
NBLK = S // SBLK      # 2 seq blocks per batch row
HB = 512              # half-block: psum-bank / matmul-N granularity
NCH = S // HB         # 4 512-chunks per row (global chunk index c)
F32 = mybir.dt.float32
BF16 = mybir.dt.bfloat16
F8 = mybir.dt.float8e4
WE_SCALE = 64.0
WH_SCALE = 512.0

_NC_CACHE = None


def _build():
    nc = bacc.Bacc(
        "TRN2", target_bir_lowering=False, debug=False, num_devices=NCORES
    )
    enc_d = nc.dram_tensor(
        "enc_t", [BC, NBLK, 128, KH, SBLK], F8, kind="ExternalInput"
    )
    hid_d = nc.dram_tensor("hid_t", [128, KD, 16], F8, kind="ExternalInput")
    wh_d = nc.dram_tensor("w_h", [128, KD, H], F8, kind="ExternalInput")
    we_d = nc.dram_tensor("w_e", [128, KH, H], F8, kind="ExternalInput")
    ba_d = nc.dram_tensor("ba4", [128, NQ, BC], F32, kind="ExternalInput")
    vsel_d = nc.dram_tensor("vsel", [128, NQ, 2, NCH, 64], BF16, kind="ExternalInput")
    m4_d = nc.dram_tensor("m4", [128, BC], F32, kind="ExternalInput")
    m4t_d = nc.dram_tensor("m4t", [BC, 128], F32, kind="ExternalInput")
    id_d = nc.dram_tensor("ident", [BC, BC], F32, kind="ExternalInput")
    out_d = nc.dram_tensor("out", [128, HB], F32, kind="ExternalOutput")

    TANH = mybir.ActivationFunctionType.Tanh
    EXP = mybir.ActivationFunctionType.Exp

    with tile.TileContext(nc) as tc:
        with (
            tc.tile_pool(name="const", bufs=1) as constp,
            tc.tile_pool(name="enc", bufs=4) as encp,
            tc.tile_pool(name="energy", bufs=3) as enp,
            tc.tile_pool(name="small", bufs=1) as smallp,
            tc.tile_pool(name="psum_e", bufs=3, space=bass.MemorySpace.PSUM) as pse,
            tc.tile_pool(name="psum_l", bufs=1, space=bass.MemorySpace.PSUM) as psl,
            tc.tile_pool(name="psum_s", bufs=1, space=bass.MemorySpace.PSUM) as pss,
        ):
            # ---- DMAs across the 3 hw queues (sync/SP, gpsimd/SWDGE, and
            # scalar for small constants that finish before the first tanh).
            # wh first on sync so h_part (the tanh bias) starts earliest. ----
            hid_sb = constp.tile([128, KD, 16], F8)
            nc.gpsimd.dma_start(hid_sb[:], hid_d[:])
            wh_sb = constp.tile([128, KD, H], F8)
            nc.sync.dma_start(wh_sb[:, 0 : KD // 2, :], wh_d[:, 0 : KD // 2, :])
            nc.gpsimd.dma_start(wh_sb[:, KD // 2 :, :], wh_d[:, KD // 2 :, :])
            we_sb = constp.tile([128, KH, H], F8)
            nc.scalar.dma_start(we_sb[:], we_d[:])
            ba_sb = constp.tile([128, NQ, BC], F32)
            nc.scalar.dma_start(ba_sb[:], ba_d[:])
            id_sb = constp.tile([BC, BC], F32)
            nc.scalar.dma_start(id_sb[:], id_d[:])
            vsel_sb = constp.tile([128, NQ, 2, NCH, 64], BF16)
            nc.scalar.dma_start(vsel_sb[:], vsel_d[:])
            m4_sb = constp.tile([128, BC], F32)
            nc.scalar.dma_start(m4_sb[:], m4_d[:])
            m4t_sb = constp.tile([BC, 128], F32)
            nc.scalar.dma_start(m4t_sb[:], m4t_d[:])

            hptb = constp.tile([128, NQ, BC], F32)
            ex = smallp.tile([128, HB], F32)
            out_sb = smallp.tile([128, HB], F32)
            esum = smallp.tile([128, 1], F32)
            rsb = smallp.tile([BC, 1], F32)
            hp_sb = smallp.tile([BC, H], F32)

            logit_ps = psl.tile([128, HB], F32)
            ps_t = pss.tile([128, HB], F32)

            # ---- PE clock warm-up on dummy data while DMAs fly ----
            warm = constp.tile([128, 512], BF16)
            nc.vector.memset(warm[:], 0.0)
            for _ in range(4):
                nc.tensor.matmul(
                    ps_t[:, :], warm[:, 0:128], warm[:], start=True, stop=True
                )

            # ---- h_part = hidden @ W_h (fp8 DoubleRow, 8 matmuls) ----
            # DR ldweights needs plane stride %16==0, so hid is padded
            # to 16 columns and hp lands on psum partitions 0:16 (4 real)
            hp_ps = ps_t[0:16, 0:H]
            for j in range(KD // 2):
                nc.tensor.matmul(
                    hp_ps,
                    hid_sb[:, 2 * j : 2 * j + 2, :],
                    wh_sb[:, 2 * j : 2 * j + 2, :],
                    start=(j == 0),
                    stop=(j == KD // 2 - 1),
                    perf_mode=mybir.MatmulPerfMode.DoubleRow,
                )
            nc.vector.tensor_copy(hp_sb[:], ps_t[0:BC, 0:H])
            # transpose to [128, q, b] via PE; fold in 1/WH_SCALE and b_attn
            for q in range(NQ):
                hpt_ps = ps_t[:, (q + 4) * BC : (q + 5) * BC]
                nc.tensor.transpose(
                    hpt_ps, hp_sb[:, q * 128 : (q + 1) * 128], id_sb[:]
                )
                nc.vector.scalar_tensor_tensor(
                    hptb[:, q, :],
                    hpt_ps,
                    1.0 / WH_SCALE,
                    ba_sb[:, q, :],
                    op0=mybir.AluOpType.mult,
                    op1=mybir.AluOpType.add,
                )

            # ---- pipeline over blocks, s-major ----
            blocks = [(b, s) for s in range(NBLK) for b in range(BC)]
            NBLOCKS = len(blocks)
            ets = {}
            epss = {}
            ens = {}

            def load_block(i):
                b, sblk = blocks[i]
                et = encp.tile([128, KH, SBLK], F8)
                nc.sync.dma_start(et[:], enc_d[b, sblk])
                ets[i] = et

            def emit_emm(i):
                et = ets[i]
                tiles = []
                for q in range(NQ):
                    tiles.append(pse.tile([128, SBLK], F32, name="eps", tag="eps"))
                for qpair in (range(0, 2), range(2, 4)):
                    for half in range(SBLK // HB):
                        hsl = slice(half * HB, (half + 1) * HB)
                        for j in range(KH // 2):
                            for q in qpair:
                                nc.tensor.matmul(
                                    tiles[q][:, hsl],
                                    we_sb[
                                        :, 2 * j : 2 * j + 2, q * 128 : (q + 1) * 128
                                    ],
                                    et[:, 2 * j : 2 * j + 2, hsl],
                                    start=(j == 0),
                                    stop=(j == KH // 2 - 1),
                                    perf_mode=mybir.MatmulPerfMode.DoubleRow,
                                )
                epss[i] = tiles
                ets.pop(i)

            def emit_tanh(i):
                b, sblk = blocks[i]
                en = enp.tile([128, NQ, SBLK], BF16)
                for q in range(NQ):
                    nc.scalar.activation(
                        en[:, q, :],
                        epss[i][q][:],
                        TANH,
                        bias=hptb[:, q, b : b + 1],
                        scale=1.0 / WE_SCALE,
                    )
                ens[i] = en
                del epss[i]

            def emit_v(i):
                # logits for (row b, global chunk c) -> psum partition 32b+c.
                # PSUM AP base partitions may only be 0/32/64, so rows pair up
                # into 64-wide strips at base 64*(b//2); the stationary column
                # 32*(b%2)+c places the v-dot on the right partition and zeros
                # elsewhere.  One accumulation group per strip: started by the
                # strip's first writer (b%2==0, chunk 0), every later matmul
                # accumulates (adding zeros outside its own column), stopped
                # by the strip's last writer (b%2==1, chunk 3).
                b, sblk = blocks[i]
                en = ens[i]
                strip = logit_ps[64 * (b // 2) : 64 * (b // 2) + 64, :]
                for half in range(SBLK // HB):
                    c = sblk * (SBLK // HB) + half
                    hsl = slice(half * HB, (half + 1) * HB)
                    for q in range(NQ):
                        nc.tensor.matmul(
                            strip,
                            vsel_sb[:, q, b % 2, c, :],
                            en[:, q, hsl],
                            start=(b % 2 == 0 and sblk == 0 and half == 0
                                   and q == 0),
                            stop=(b % 2 == 1 and sblk == NBLK - 1
                                  and half == SBLK // HB - 1 and q == NQ - 1),
                        )
                del ens[i]

            load_block(0)
            load_block(1)
            emit_emm(0)
            emit_tanh(0)
            for i in range(1, NBLOCKS):
                if i + 1 < NBLOCKS:
                    load_block(i + 1)
                emit_emm(i)
                emit_tanh(i)
                emit_v(i - 1)
            emit_v(NBLOCKS - 1)

            # ---- softmax tail ----
            nc.scalar.activation(ex[:], logit_ps[:], EXP, accum_out=esum[:])
            rs4_ps = ps_t[0:BC, 0:1]
            nc.tensor.matmul(rs4_ps, m4_sb[:], esum[:], start=True, stop=True)
            nc.vector.reciprocal(rsb[:], rs4_ps)
            rsB_ps = ps_t[:, 4:5]
            nc.tensor.matmul(rsB_ps, m4t_sb[:], rsb[:], start=True, stop=True)
            nc.vector.tensor_scalar_mul(out_sb[:], ex[:], rsB_ps)
            nc.sync.dma_start(out_d[:], out_sb[:])

    nc.compile()
    return nc


def _get_nc():
    global _NC_CACHE
    if _NC_CACHE is None:
        _NC_CACHE = _build()
    return _NC_CACHE


def _prep_inputs(hidden, encoder_outputs, W_attn, b_attn, v):
    f = np.float32
    W_h = np.asarray(W_attn[:DH], dtype=f)
    W_e = np.asarray(W_attn[DH:], dtype=f)
    import ml_dtypes
    bf = ml_dtypes.bfloat16
    f8 = ml_dtypes.float8_e4m3
    wh_prep = np.clip(
        np.ascontiguousarray(W_h.reshape(KD, 128, H).transpose(1, 0, 2)) * WH_SCALE,
        -240.0, 240.0,
    ).astype(f8)
    we_prep = np.clip(
        np.ascontiguousarray(W_e.reshape(KH, 128, H).transpose(1, 0, 2)) * WE_SCALE,
        -240.0, 240.0,
    ).astype(f8)
    b_attn = np.asarray(b_attn, dtype=f)
    v = np.asarray(v, dtype=f)
    # ba4[p, q, j] = b_attn[q*128+p] replicated over the BC free dim
    ba_prep = np.ascontiguousarray(
        np.broadcast_to(b_attn.reshape(NQ, 128).T[:, :, None], (128, NQ, BC))
    ).astype(f)
    # vsel[p, q, b01, c, j] = v[q*128+p] if j == 32*b01 + c else 0
    vq = v.reshape(NQ, 128).T                      # [128, NQ]
    vsel = np.zeros((128, NQ, 2, NCH, 64), dtype=f)
    for b01 in range(2):
        for c in range(NCH):
            vsel[:, :, b01, c, 32 * b01 + c] = vq
    vsel_prep = vsel.astype(bf)
    # m4[32b+c, b] = 1 ; m4t[b, 32b+c] = 1
    m4 = np.zeros((128, BC), dtype=f)
    m4t = np.zeros((BC, 128), dtype=f)
    for b in range(BC):
        for c in range(NCH):
            m4[32 * b + c, b] = 1.0
            m4t[b, 32 * b + c] = 1.0
    ident = np.eye(BC, dtype=f)
    hidden = np.asarray(hidden, dtype=f)
    encoder_outputs = np.asarray(encoder_outputs, dtype=f)

    in_maps = []
    for core in range(NCORES):
        b0 = core * BC
        hc = hidden[b0 : b0 + BC]                       # [BC, DH]
        hid_prep = np.zeros((128, KD, 16), dtype=f8)
        hid_prep[:, :, 0:BC] = np.clip(
            np.ascontiguousarray(hc.T.reshape(KD, 128, BC).transpose(1, 0, 2)),
            -240.0, 240.0,
        ).astype(f8)
        ec = encoder_outputs[:, b0 : b0 + BC, :]        # [S, BC, H]
        # enc_prep[b, sblk, p, k, si] = ec[sblk*SBLK+si, b, k*128+p]
        enc_prep = np.clip(
            np.ascontiguousarray(
                ec.transpose(1, 0, 2)
                .reshape(BC, NBLK, SBLK, KH, 128)
                .transpose(0, 1, 4, 3, 2)
            ),
            -240.0, 240.0,
        ).astype(f8)
        in_maps.append(
            {
                "enc_t": enc_prep,
                "hid_t": hid_prep,
                "w_h": wh_prep,
                "w_e": we_prep,
                "ba4": ba_prep,
                "vsel": vsel_prep,
                "m4": m4,
                "m4t": m4t,
                "ident": ident,
            }
        )
    return in_maps


def _run(inputs, trace=False, **kw):
    nc = _get_nc()
    in_maps = _prep_inputs(
        inputs["hidden"],
        inputs["encoder_outputs"],
        inputs["W_attn"],
        inputs["b_attn"],
        inputs["v"],
    )
    res = run_bass_kernel_spmd(
        nc, in_maps, core_ids=list(range(NCORES)), trace=trace, **kw
    )
    # out_dev[32b+c, :] holds out[b, 512c : 512(c+1)]
    pieces = []
    for r in res.results:
        od = np.asarray(r["out"], dtype=np.float32)     # [128, 512]
        rows = od.reshape(BC, 32, HB)[:, 0:NCH, :]      # [BC, NCH, 512]
        pieces.append(rows.reshape(BC, S))
    out = np.concatenate(pieces, axis=0).astype(np.float32)
    return out, res


def kernel(**inputs):
    out, _ = _run(inputs, trace=False)
    return out
